# revision 1
# baseline (speedup 1.0000x reference)
"""nn_Attention_21285857919430: GroupNorm + single-head attention block.

Strategy: data-parallel over batch across the 8 NeuronCores (2 samples per
core); the small (C,C) weights are replicated to every core. Math per sample
is identical to the reference (including the faithful non-transposing
reshape of the (b*n, c) projection buffers back to (b, c, n)).

All shapes are hardcoded per the problem spec:
  x: (16, 512, 64, 64) f32, weights (512,512), biases (512,)
"""

import numpy as np
import jax
import jax.numpy as jnp

B, C, H, W = 16, 512, 64, 64
G = 32
EPS = 1e-5
N_CORES = 8

_WEIGHT_KEYS = ["gn_w", "gn_b", "Wq", "bq", "Wk", "bk", "Wv", "bv", "Wo", "bo"]


def _forward(x, gn_w, gn_b, Wq, bq, Wk, bk, Wv, bv, Wo, bo):
    # x: (local_b, C, H, W) — per-core shard of the batch.
    #
    # Transpose-light formulation. The reference computes qf = xnf @ Wq.T on
    # the (b*n, c) transposed activations, then REINTERPRETS that buffer as
    # (b, c, n) row-major. Algebraically, with qfT = Wq @ xn + bq (shape
    # (c, n), no big transpose needed) and n split as (j1=8, j0=512):
    #   q(c-major)[i, j1*512+j0] = qfT[j0, i*8+j1]
    # so scores/att become multi-dim dot_general contractions over the
    # reshaped qfT/kfT/vfT buffers — XLA contracts them directly instead of
    # materializing (b,n,c) transposes through DVE transpose kernels. Only
    # the single final ofT -> of transpose is materialized.
    b, c, h, w = x.shape
    n = h * w
    x3 = x.reshape(b, c, n)
    xg = x3.reshape(b, G, (c // G) * n)
    mu = xg.mean(-1, keepdims=True)
    var = xg.var(-1, keepdims=True)
    xn = ((xg - mu) / jnp.sqrt(var + EPS)).reshape(b, c, n)
    xn = xn * gn_w[None, :, None] + gn_b[None, :, None]
    # Matmuls run with bf16 operands + fp32 accumulation (4x PE rate vs
    # fp32's 4-cycles-per-row); GroupNorm, softmax, bias adds, and the
    # residual stay fp32.
    bf = jnp.bfloat16
    f32 = jnp.float32
    xnb = xn.astype(bf)
    qfT = jnp.einsum("cd,bdn->bcn", Wq.astype(bf), xnb,
                     preferred_element_type=f32) + bq[None, :, None]
    kfT = jnp.einsum("cd,bdn->bcn", Wk.astype(bf), xnb,
                     preferred_element_type=f32) + bk[None, :, None]
    vfT = jnp.einsum("cd,bdn->bcn", Wv.astype(bf), xnb,
                     preferred_element_type=f32) + bv[None, :, None]
    qfT4 = qfT.reshape(b, c, c, 8)  # (b, j0, i, j1)
    kfT4 = kfT.reshape(b, c, c, 8)
    vfT4 = vfT.reshape(b, c, c, 8)
    scale = 1.0 / jnp.sqrt(jnp.float32(c))
    scores = jnp.einsum("bkcj,bkdj->bcd", qfT4.astype(bf), kfT4.astype(bf),
                        preferred_element_type=f32) * scale
    weights = jax.nn.softmax(scores, axis=-1)
    att = jnp.einsum("bcd,bkdj->bcjk", weights.astype(bf), vfT4.astype(bf),
                     preferred_element_type=f32).reshape(b, c, n)
    ofT = jnp.einsum("ce,ben->bcn", Wo.astype(bf), att.astype(bf),
                     preferred_element_type=f32) + bo[None, :, None]
    out = ofT.transpose(0, 2, 1).reshape(b, c, n)
    return (x3 + out).reshape(b, c, h, w)


_pmapped = jax.pmap(_forward, in_axes=(0,) + (None,) * 10)


def kernel(**inputs) -> np.ndarray:
    x = np.asarray(inputs["x"], dtype=np.float32)
    shard = B // N_CORES  # 2 samples per core
    xs = x.reshape(N_CORES, shard, C, H, W)
    rest = [np.asarray(inputs[k], dtype=np.float32) for k in _WEIGHT_KEYS]
    out = _pmapped(xs, *rest)
    return np.asarray(out).reshape(B, C, H, W).astype(np.float32)


if __name__ == "__main__":
    rng = np.random.default_rng(0)
    demo = {
        "x": rng.standard_normal((B, C, H, W), dtype=np.float32),
        "gn_w": np.ones((C,), np.float32),
        "gn_b": np.zeros((C,), np.float32),
    }
    for nm in ["Wq", "Wk", "Wv", "Wo"]:
        demo[nm] = (rng.standard_normal((C, C)) * 0.02).astype(np.float32)
    for nm in ["bq", "bk", "bv", "bo"]:
        demo[nm] = (rng.standard_normal((C,)) * 0.02).astype(np.float32)
    y = kernel(**demo)
    print("ok", y.shape, y.dtype)



# revision 4
# speedup vs baseline: 1.0758x; 1.0758x over previous
"""nn_Attention Bass/Tile kernel: GroupNorm + single-head attention block.

Data-parallel over batch across 8 NeuronCores (2 samples per core); the
(C,C) weights are replicated. Per sample the reference math (including the
non-transposing (b*n,c)->(b,c,n) buffer reinterpretations) is reorganized so
that *every* contraction runs on the tensor engine with contiguous operands
and no explicit transposes:

  - x (c,n) -> GroupNorm -> xn stored PERMUTED: xn_p[c, (n%8)*512 + n//8]
  - Q/K projections read xn_p chunks (moving) with W^T stationary -> qf_p/kf_p
    land j1-major: qf_p[j0, j1*512+c] = qfT[j0, 8c+j1]
  - scores are computed transposed: ST[d,c] += kf_p_blk^T @ qf_p_blk
    (contraction over the j0 partition axis, per j1 block)
  - softmax denominators via an all-ones matmul over exp(ST) columns;
    the 1/rowsum scale is deferred to the att PSUM->SBUF copy
  - V projection emits Vl[j1][d,j0] directly (xn_p strided... no: contiguous
    slices as stationary, W_v^T moving); the +bv bias is one extra matmul
    with an all-ones stationary against a broadcast bias tile
  - att[e, m] accumulates from expST (stationary) x Vl (moving); the
    PSUM->SBUF copy applies the softmax 1/sum scale and scatters columns to
    att_pp[e, (m%8)*512 + m//8]
  - out projection consumes att_pp slices as stationary against W_o^T moving,
    yielding D[c, j1*512+j0] chunks directly in the output layout; +bo via the
    ones-matmul trick; residual add fuses the PSUM read with the x re-read.

All shapes hardcoded: x (16,512,64,64) f32, weights (512,512), biases (512,).
"""

import numpy as np
import ml_dtypes

import concourse.bacc as bacc
import concourse.tile as tile
from concourse import mybir
from concourse.bass_utils import run_bass_kernel_spmd

B, C, H, W = 16, 512, 64, 64
N = H * W  # 4096
G = 32
EPS = 1e-5
N_CORES = 8
SB = B // N_CORES  # samples per core
NT = C // 128  # 4 channel tiles
F32 = mybir.dt.float32
BF16 = mybir.dt.bfloat16
BF_NP = ml_dtypes.bfloat16
SCALE = float(1.0 / np.sqrt(np.float32(C)))

_WEIGHT_KEYS = ["gn_w", "gn_b", "Wq", "bq", "Wk", "bk", "Wv", "bv", "Wo", "bo"]

# ---------------------------------------------------------------------------
# Bass program
# ---------------------------------------------------------------------------

_NC_CACHE = {}


def _emit(nc, tc, aps, pools):
    x_ap = aps["x"]
    y_ap = aps["y"]
    const, xpool, big, expool, spool, rsp, xrp, yp, pmm, psm = pools

    # ---- constants into SBUF ----
    wsb = {}
    for wname in ("wq_t", "wk_t", "wv_t", "wo_t"):
        t = const.tile([128, NT, C], BF16, tag=wname)
        nc.sync.dma_start(out=t, in_=aps[wname].rearrange("(kt p) i -> p kt i", p=128))
        wsb[wname] = t
    vec_sb = {}
    for vname in ("bq", "bk", "gn_w", "gn_b"):
        t = const.tile([128, NT], F32, tag=vname)
        nc.sync.dma_start(out=t, in_=aps[vname].rearrange("(t p) -> p t", p=128))
        vec_sb[vname] = t
    ones_sb = const.tile([128, 128], BF16, tag="ones")
    nc.sync.dma_start(out=ones_sb, in_=aps["ones"])
    bd_sb = const.tile([128, 128], F32, tag="blkdiag")
    nc.sync.dma_start(out=bd_sb, in_=aps["blkdiag"])
    bvb_sb = const.tile([128, C], BF16, tag="bvb")
    nc.sync.dma_start(out=bvb_sb, in_=aps["bvb"])
    bob_sb = const.tile([128, C], BF16, tag="bob")
    nc.sync.dma_start(out=bob_sb, in_=aps["bob"])
    eps_sb = const.tile([128, 1], F32, tag="eps")
    nc.vector.memset(eps_sb, EPS)

    for s in range(SB):
        # ------------------- GroupNorm -> xn_p (permuted bf16) -------------
        xnp = []
        for ct in range(NT):
            xt = xpool.tile([128, N], F32, tag="x")
            nc.sync.dma_start(out=xt, in_=x_ap[s, 128 * ct : 128 * (ct + 1), :])
            stats = spool.tile([128, 8, 6], F32, tag="stats")
            for u in range(8):
                nc.vector.bn_stats(out=stats[:, u, :], in_=xt[:, 512 * u : 512 * (u + 1)])
            mv = spool.tile([128, 2], F32, tag="mv")
            nc.vector.bn_aggr(out=mv, in_=stats)
            # mvfix = [mean_i, var_i + mean_i^2]
            mvfix = spool.tile([128, 2], F32, tag="mvfix")
            t1 = spool.tile([128, 1], F32, tag="t1")
            nc.vector.tensor_copy(out=mvfix[:, 0:1], in_=mv[:, 0:1])
            nc.vector.tensor_mul(out=t1, in0=mv[:, 0:1], in1=mv[:, 0:1])
            nc.vector.tensor_add(out=mvfix[:, 1:2], in0=mv[:, 1:2], in1=t1)
            # group aggregate broadcast: gsum[p,:] = (1/16)*sum over p's group
            ps2 = psm.tile([128, 2], F32, tag="small")
            nc.tensor.matmul(ps2, lhsT=bd_sb, rhs=mvfix, start=True, stop=True)
            gsub = spool.tile([128, 2], F32, tag="gsub")
            nc.scalar.copy(out=gsub, in_=ps2)
            mu2 = spool.tile([128, 1], F32, tag="mu2")
            nc.vector.tensor_mul(out=mu2, in0=gsub[:, 0:1], in1=gsub[:, 0:1])
            varg = spool.tile([128, 1], F32, tag="varg")
            nc.vector.tensor_tensor(
                out=varg, in0=gsub[:, 1:2], in1=mu2, op=mybir.AluOpType.subtract
            )
            sd = spool.tile([128, 1], F32, tag="sd")
            nc.scalar.activation(
                out=sd, in_=varg, func=mybir.ActivationFunctionType.Sqrt,
                bias=eps_sb, scale=1.0,
            )
            rstd = spool.tile([128, 1], F32, tag="rstd")
            nc.vector.reciprocal(out=rstd, in_=sd)
            A = spool.tile([128, 1], F32, tag="A")
            nc.vector.tensor_mul(out=A, in0=rstd, in1=vec_sb["gn_w"][:, ct : ct + 1])
            t2 = spool.tile([128, 1], F32, tag="t2")
            nc.vector.tensor_mul(out=t2, in0=gsub[:, 0:1], in1=A)
            Bv = spool.tile([128, 1], F32, tag="Bv")
            nc.vector.tensor_tensor(
                out=Bv, in0=vec_sb["gn_b"][:, ct : ct + 1], in1=t2,
                op=mybir.AluOpType.subtract,
            )
            xnt = big.tile([128, 8, 512], BF16, tag="big")
            nc.vector.tensor_scalar(
                out=xnt,
                in0=xt.rearrange("p (a r) -> p r a", r=8),
                scalar1=A, scalar2=Bv,
                op0=mybir.AluOpType.mult, op1=mybir.AluOpType.add,
            )
            xnp.append(xnt)

        # ------------------- Q/K projections (permuted layout) -------------
        qfp, kfp = [], []
        for wname, bname, dest in (("wq_t", "bq", qfp), ("wk_t", "bk", kfp)):
            for it in range(NT):
                dt_ = big.tile([128, 8, 512], BF16, tag="big")
                dest.append(dt_)
                for g in range(2):
                    pss = [pmm.tile([128, 512], F32, tag="mm", name=f"mmps{g}_{i_}") for i_ in range(4)]
                    for kt in range(NT):
                        for jj in range(4):
                            w = 4 * g + jj
                            nc.tensor.matmul(
                                pss[jj],
                                lhsT=wsb[wname][:, kt, 128 * it : 128 * (it + 1)],
                                rhs=xnp[kt][:, w, :],
                                start=(kt == 0), stop=(kt == NT - 1),
                            )
                    for jj in range(4):
                        w = 4 * g + jj
                        nc.scalar.activation(
                            out=dt_[:, w, :], in_=pss[jj],
                            func=mybir.ActivationFunctionType.Identity,
                            bias=vec_sb[bname][:, it : it + 1], scale=1.0,
                        )

        # ------------------- scores (transposed) + exp ---------------------
        expst = []
        for dt in range(NT):
            ps = pmm.tile([128, 512], F32, tag="mm")
            idx = 0
            for j1 in range(8):
                for jt in range(NT):
                    nc.tensor.matmul(
                        ps,
                        lhsT=kfp[jt][:, j1, 128 * dt : 128 * (dt + 1)],
                        rhs=qfp[jt][:, j1, :],
                        start=(idx == 0), stop=(idx == 31),
                    )
                    idx += 1
            et_ = expool.tile([128, 512], BF16, tag="expst")
            nc.scalar.activation(
                out=et_, in_=ps, func=mybir.ActivationFunctionType.Exp, scale=SCALE
            )
            expst.append(et_)

        # ------------------- softmax denominators -> r ---------------------
        rsps = psm.tile([128, 512], F32, tag="small")
        for dt in range(NT):
            nc.tensor.matmul(
                rsps, lhsT=ones_sb, rhs=expst[dt], start=(dt == 0), stop=(dt == NT - 1)
            )
        rs_sb = rsp.tile([128, 512], F32, tag="rs")
        nc.scalar.copy(out=rs_sb, in_=rsps)
        rcol = rsp.tile([128, 4], F32, tag="rcol")
        for et in range(NT):
            nc.sync.dma_start(
                out=rcol[:, et : et + 1], in_=rs_sb[0:1, 128 * et : 128 * (et + 1)]
            )
        nc.vector.reciprocal(out=rcol, in_=rcol)

        # ------------------- V projection -> Vl ----------------------------
        vl = []
        for tt in range(NT):
            vt = big.tile([128, 8, 512], BF16, tag="big")
            vl.append(vt)
            for j1 in range(8):
                ps = pmm.tile([128, 512], F32, tag="mm")
                for kt in range(NT):
                    nc.tensor.matmul(
                        ps,
                        lhsT=xnp[kt][:, j1, 128 * tt : 128 * (tt + 1)],
                        rhs=wsb["wv_t"][:, kt, :],
                        start=(kt == 0), stop=False,
                    )
                nc.tensor.matmul(ps, lhsT=ones_sb, rhs=bvb_sb, start=False, stop=True)
                nc.scalar.copy(out=vt[:, j1, :], in_=ps)

        # ------------------- attention applied to V ------------------------
        attp = []
        for et in range(NT):
            at_ = big.tile([128, 8, 512], BF16, tag="big")
            attp.append(at_)
            for g in range(2):
                pss = [pmm.tile([128, 512], F32, tag="mm", name=f"mmps{g}_{i_}") for i_ in range(4)]
                for dt in range(NT):
                    for jj in range(4):
                        j1v = 4 * g + jj
                        nc.tensor.matmul(
                            pss[jj],
                            lhsT=expst[dt][:, 128 * et : 128 * (et + 1)],
                            rhs=vl[dt][:, j1v, :],
                            start=(dt == 0), stop=(dt == NT - 1),
                        )
                for jj in range(4):
                    j1v = 4 * g + jj
                    nc.scalar.activation(
                        out=at_[:, :, 64 * j1v : 64 * j1v + 64],
                        in_=pss[jj].rearrange("p (a r) -> p r a", r=8),
                        func=mybir.ActivationFunctionType.Copy,
                        scale=rcol[:, et : et + 1],
                    )

        # ------------------- out projection + residual ---------------------
        for ct in range(NT):
            for g in range(2):
                pss = [pmm.tile([128, 512], F32, tag="mm", name=f"mmps{g}_{i_}") for i_ in range(4)]
                for jj in range(4):
                    j1 = 4 * g + jj
                    for et in range(NT):
                        nc.tensor.matmul(
                            pss[jj],
                            lhsT=attp[et][:, j1, 128 * ct : 128 * (ct + 1)],
                            rhs=wsb["wo_t"][:, et, :],
                            start=(et == 0), stop=False,
                        )
                    nc.tensor.matmul(
                        pss[jj], lhsT=ones_sb, rhs=bob_sb, start=False, stop=True
                    )
                for jj in range(4):
                    j1 = 4 * g + jj
                    xr = xrp.tile([128, 512], F32, tag="xr")
                    nc.sync.dma_start(
                        out=xr,
                        in_=x_ap[s, 128 * ct : 128 * (ct + 1), 512 * j1 : 512 * (j1 + 1)],
                    )
                    yst = yp.tile([128, 512], F32, tag="y")
                    nc.vector.tensor_add(out=yst, in0=pss[jj], in1=xr)
                    nc.sync.dma_start(
                        out=y_ap[s, 128 * ct : 128 * (ct + 1), 512 * j1 : 512 * (j1 + 1)],
                        in_=yst,
                    )


def build_nc():
    if "nc" in _NC_CACHE:
        return _NC_CACHE["nc"]
    nc = bacc.Bacc("TRN2", target_bir_lowering=False, debug=False, num_devices=N_CORES)
    aps = {}
    aps["x"] = nc.dram_tensor("x", [SB, C, N], F32, kind="ExternalInput").ap()
    aps["y"] = nc.dram_tensor("y", [SB, C, N], F32, kind="ExternalOutput").ap()
    for wname in ("wq_t", "wk_t", "wv_t", "wo_t"):
        aps[wname] = nc.dram_tensor(wname, [C, C], BF16, kind="ExternalInput").ap()
    for vname in ("bq", "bk", "gn_w", "gn_b"):
        aps[vname] = nc.dram_tensor(vname, [C], F32, kind="ExternalInput").ap()
    aps["ones"] = nc.dram_tensor("ones", [128, 128], BF16, kind="ExternalInput").ap()
    aps["blkdiag"] = nc.dram_tensor("blkdiag", [128, 128], F32, kind="ExternalInput").ap()
    aps["bvb"] = nc.dram_tensor("bvb", [128, C], BF16, kind="ExternalInput").ap()
    aps["bob"] = nc.dram_tensor("bob", [128, C], BF16, kind="ExternalInput").ap()

    with tile.TileContext(nc) as tc:
        with (
            tc.tile_pool(name="const", bufs=1) as const,
            tc.tile_pool(name="xpool", bufs=2) as xpool,
            tc.tile_pool(name="big", bufs=16) as big,
            tc.tile_pool(name="expool", bufs=6) as expool,
            tc.tile_pool(name="spool", bufs=4) as spool,
            tc.tile_pool(name="rsp", bufs=2) as rsp,
            tc.tile_pool(name="xrp", bufs=3) as xrp,
            tc.tile_pool(name="yp", bufs=3) as yp,
            tc.tile_pool(name="pmm", bufs=6, space="PSUM") as pmm,
            tc.tile_pool(name="psm", bufs=2, space="PSUM") as psm,
        ):
            _emit(nc, tc, aps, (const, xpool, big, expool, spool, rsp, xrp, yp, pmm, psm))
    nc.compile()
    _NC_CACHE["nc"] = nc
    return nc


# ---------------------------------------------------------------------------
# host side
# ---------------------------------------------------------------------------


def make_const_inputs(inputs):
    f32 = np.float32
    out = {
        "wq_t": np.ascontiguousarray(np.asarray(inputs["Wq"], f32).T).astype(BF_NP),
        "wk_t": np.ascontiguousarray(np.asarray(inputs["Wk"], f32).T).astype(BF_NP),
        "wv_t": np.ascontiguousarray(np.asarray(inputs["Wv"], f32).T).astype(BF_NP),
        "wo_t": np.ascontiguousarray(np.asarray(inputs["Wo"], f32).T).astype(BF_NP),
        "bq": np.asarray(inputs["bq"], f32),
        "bk": np.asarray(inputs["bk"], f32),
        "gn_w": np.asarray(inputs["gn_w"], f32),
        "gn_b": np.asarray(inputs["gn_b"], f32),
        "ones": np.ones((128, 128), BF_NP),
        "blkdiag": (np.kron(np.eye(8, dtype=f32), np.ones((16, 16), f32)) / 16.0).astype(f32),
        "bvb": np.ascontiguousarray(
            np.broadcast_to((np.asarray(inputs["bv"], f32) / 128.0).astype(BF_NP), (128, C))
        ),
        "bob": np.ascontiguousarray(
            np.broadcast_to((np.asarray(inputs["bo"], f32) / 128.0).astype(BF_NP), (128, C))
        ),
    }
    return out


def make_in_maps(inputs):
    x = np.asarray(inputs["x"], np.float32).reshape(B, C, N)
    const = make_const_inputs(inputs)
    in_maps = []
    for i in range(N_CORES):
        m = dict(const)
        m["x"] = np.ascontiguousarray(x[SB * i : SB * (i + 1)])
        in_maps.append(m)
    return in_maps


def kernel(**inputs) -> np.ndarray:
    nc = build_nc()
    in_maps = make_in_maps(inputs)
    res = run_bass_kernel_spmd(nc, in_maps, list(range(N_CORES)))
    out = np.empty((B, C, N), np.float32)
    for i in range(N_CORES):
        out[SB * i : SB * (i + 1)] = res.results[i]["y"]
    return out.reshape(B, C, H, W)


# ---------------------------------------------------------------------------
# cached PJRT runner (for repeat calls / on-device timing)
# ---------------------------------------------------------------------------


def pjrt_runner():
    """Build (once) a jitted 8-core runner over the compiled Bass module.

    Returns (fn, in_names, out_names). fn takes concat-along-axis-0 arrays
    (one per in_name, then one zero buffer per out_name) and returns concat
    outputs. Mirrors bass2jax.run_bass_via_pjrt but caches the jit so repeat
    calls skip re-tracing.
    """
    if "runner" in _NC_CACHE:
        return _NC_CACHE["runner"]
    import jax
    from jax.sharding import Mesh, PartitionSpec
    from jax.experimental.shard_map import shard_map
    from concourse import bass2jax

    bass2jax.install_neuronx_cc_hook()
    nc = build_nc()
    assert nc.dbg_addr is None
    partition_name = nc.partition_id_tensor.name if nc.partition_id_tensor else None
    in_names, out_names, out_avals = [], [], []
    for alloc in nc.m.functions[0].allocations:
        if not isinstance(alloc, mybir.MemoryLocationSet):
            continue
        name = alloc.memorylocations[0].name
        if alloc.kind == "ExternalInput":
            if name != partition_name:
                in_names.append(name)
        elif alloc.kind == "ExternalOutput":
            out_names.append(name)
            out_avals.append(
                jax.core.ShapedArray(tuple(alloc.tensor_shape), mybir.dt.np(alloc.dtype))
            )

    all_names = tuple(in_names) + tuple(out_names)
    if partition_name is not None:
        all_names = all_names + (partition_name,)

    def _body(*args):
        operands = list(args)
        if partition_name is not None:
            operands.append(bass2jax.partition_id_tensor())
        outs = bass2jax._bass_exec_p.bind(
            *operands,
            out_avals=tuple(out_avals),
            in_names=all_names,
            out_names=tuple(out_names),
            lowering_input_output_aliases=(),
            sim_require_finite=True,
            sim_require_nnan=True,
            nc=nc,
        )
        return tuple(outs)

    devices = jax.devices()[:N_CORES]
    mesh = Mesh(np.asarray(devices), ("core",))
    nargs = len(in_names) + len(out_names)
    fn = jax.jit(
        shard_map(
            _body,
            mesh=mesh,
            in_specs=(PartitionSpec("core"),) * nargs,
            out_specs=(PartitionSpec("core"),) * len(out_names),
            check_rep=False,
        ),
        keep_unused=True,
    )
    _NC_CACHE["runner"] = (fn, in_names, out_names, out_avals, mesh)
    return _NC_CACHE["runner"]


if __name__ == "__main__":
    rng = np.random.default_rng(0)
    demo = {
        "x": rng.standard_normal((B, C, H, W), dtype=np.float32),
        "gn_w": np.ones((C,), np.float32),
        "gn_b": np.zeros((C,), np.float32),
    }
    for nm in ["Wq", "Wk", "Wv", "Wo"]:
        demo[nm] = (rng.standard_normal((C, C)) * 0.02).astype(np.float32)
    for nm in ["bq", "bk", "bv", "bo"]:
        demo[nm] = (rng.standard_normal((C,)) * 0.02).astype(np.float32)
    y = kernel(**demo)
    print("ok", y.shape, y.dtype)


# revision 7
# speedup vs baseline: 179.6162x; 166.9573x over previous
"""nn_Attention Bass/Tile kernel: GroupNorm + single-head attention block.

Data-parallel over batch across 8 NeuronCores (2 samples per core); the
(C,C) weights are replicated. Per sample the reference math (including the
non-transposing (b*n,c)->(b,c,n) buffer reinterpretations) is reorganized so
that *every* contraction runs on the tensor engine with contiguous operands
and no explicit transposes:

  - x (c,n) -> GroupNorm -> xn stored PERMUTED: xn_p[c, (n%8)*512 + n//8]
  - Q/K projections read xn_p chunks (moving) with W^T stationary -> qf_p/kf_p
    land j1-major: qf_p[j0, j1*512+c] = qfT[j0, 8c+j1]
  - scores are computed transposed: ST[d,c] += kf_p_blk^T @ qf_p_blk
    (contraction over the j0 partition axis, per j1 block)
  - softmax denominators via an all-ones matmul over exp(ST) columns;
    the 1/rowsum scale is deferred to the att PSUM->SBUF copy
  - V projection emits Vl[j1][d,j0] directly (xn_p strided... no: contiguous
    slices as stationary, W_v^T moving); the +bv bias is one extra matmul
    with an all-ones stationary against a broadcast bias tile
  - att[e, m] accumulates from expST (stationary) x Vl (moving); the
    PSUM->SBUF copy applies the softmax 1/sum scale and scatters columns to
    att_pp[e, (m%8)*512 + m//8]
  - out projection consumes att_pp slices as stationary against W_o^T moving,
    yielding D[c, j1*512+j0] chunks directly in the output layout; +bo via the
    ones-matmul trick; residual add fuses the PSUM read with the x re-read.

All shapes hardcoded: x (16,512,64,64) f32, weights (512,512), biases (512,).
"""

import numpy as np
import ml_dtypes

import concourse.bacc as bacc
import concourse.tile as tile
from concourse import mybir
from concourse.bass_utils import run_bass_kernel_spmd

B, C, H, W = 16, 512, 64, 64
N = H * W  # 4096
G = 32
EPS = 1e-5
N_CORES = 8
SB = B // N_CORES  # samples per core
NT = C // 128  # 4 channel tiles
F32 = mybir.dt.float32
BF16 = mybir.dt.bfloat16
BF_NP = ml_dtypes.bfloat16
SCALE = float(1.0 / np.sqrt(np.float32(C)))

_WEIGHT_KEYS = ["gn_w", "gn_b", "Wq", "bq", "Wk", "bk", "Wv", "bv", "Wo", "bo"]

# ---------------------------------------------------------------------------
# Bass program
# ---------------------------------------------------------------------------

_NC_CACHE = {}


def _emit(nc, tc, aps, pools):
    x_ap = aps["x"]
    y_ap = aps["y"]
    const, xpool, big, expool, spool, rsp, xrp, yp, pmm, psm = pools

    # ---- constants into SBUF ----
    wsb = {}
    for wname in ("wq_t", "wk_t", "wv_t", "wo_t"):
        t = const.tile([128, NT, C], BF16, tag=wname)
        nc.sync.dma_start(out=t, in_=aps[wname].rearrange("(kt p) i -> p kt i", p=128))
        wsb[wname] = t
    vec_sb = {}
    for vname in ("bq", "bk", "gn_w", "gn_b"):
        t = const.tile([128, NT], F32, tag=vname)
        nc.sync.dma_start(out=t, in_=aps[vname].rearrange("(t p) -> p t", p=128))
        vec_sb[vname] = t
    ones_sb = const.tile([128, 128], BF16, tag="ones")
    nc.sync.dma_start(out=ones_sb, in_=aps["ones"])
    bd_sb = const.tile([128, 128], F32, tag="blkdiag")
    nc.sync.dma_start(out=bd_sb, in_=aps["blkdiag"])
    bvb_sb = const.tile([128, C], BF16, tag="bvb")
    nc.sync.dma_start(out=bvb_sb, in_=aps["bvb"])
    bob_sb = const.tile([128, C], BF16, tag="bob")
    nc.sync.dma_start(out=bob_sb, in_=aps["bob"])
    eps_sb = const.tile([128, 1], F32, tag="eps")
    nc.vector.memset(eps_sb, EPS)

    for s in range(SB):
        # ------------------- GroupNorm -> xn_p (permuted bf16) -------------
        xnp = []
        for ct in range(NT):
            xt = xpool.tile([128, N], F32, tag="x")
            nc.sync.dma_start(out=xt, in_=x_ap[s, 128 * ct : 128 * (ct + 1), :])
            stats = spool.tile([128, 8, 6], F32, tag="stats")
            for u in range(8):
                nc.vector.bn_stats(out=stats[:, u, :], in_=xt[:, 512 * u : 512 * (u + 1)])
            mv = spool.tile([128, 2], F32, tag="mv")
            nc.vector.bn_aggr(out=mv, in_=stats)
            # mvfix = [mean_i, var_i + mean_i^2]
            mvfix = spool.tile([128, 2], F32, tag="mvfix")
            t1 = spool.tile([128, 1], F32, tag="t1")
            nc.vector.tensor_copy(out=mvfix[:, 0:1], in_=mv[:, 0:1])
            nc.vector.tensor_mul(out=t1, in0=mv[:, 0:1], in1=mv[:, 0:1])
            nc.vector.tensor_add(out=mvfix[:, 1:2], in0=mv[:, 1:2], in1=t1)
            # group aggregate broadcast: gsum[p,:] = (1/16)*sum over p's group
            ps2 = psm.tile([128, 2], F32, tag="small")
            nc.tensor.matmul(ps2, lhsT=bd_sb, rhs=mvfix, start=True, stop=True)
            gsub = spool.tile([128, 2], F32, tag="gsub")
            nc.scalar.copy(out=gsub, in_=ps2)
            mu2 = spool.tile([128, 1], F32, tag="mu2")
            nc.vector.tensor_mul(out=mu2, in0=gsub[:, 0:1], in1=gsub[:, 0:1])
            varg = spool.tile([128, 1], F32, tag="varg")
            nc.vector.tensor_tensor(
                out=varg, in0=gsub[:, 1:2], in1=mu2, op=mybir.AluOpType.subtract
            )
            sd = spool.tile([128, 1], F32, tag="sd")
            nc.scalar.activation(
                out=sd, in_=varg, func=mybir.ActivationFunctionType.Sqrt,
                bias=eps_sb, scale=1.0,
            )
            rstd = spool.tile([128, 1], F32, tag="rstd")
            nc.vector.reciprocal(out=rstd, in_=sd)
            A = spool.tile([128, 1], F32, tag="A")
            nc.vector.tensor_mul(out=A, in0=rstd, in1=vec_sb["gn_w"][:, ct : ct + 1])
            t2 = spool.tile([128, 1], F32, tag="t2")
            nc.vector.tensor_mul(out=t2, in0=gsub[:, 0:1], in1=A)
            Bv = spool.tile([128, 1], F32, tag="Bv")
            nc.vector.tensor_tensor(
                out=Bv, in0=vec_sb["gn_b"][:, ct : ct + 1], in1=t2,
                op=mybir.AluOpType.subtract,
            )
            xnt = big.tile([128, 8, 512], BF16, tag="big")
            nc.vector.tensor_scalar(
                out=xnt,
                in0=xt.rearrange("p (a r) -> p r a", r=8),
                scalar1=A, scalar2=Bv,
                op0=mybir.AluOpType.mult, op1=mybir.AluOpType.add,
            )
            xnp.append(xnt)

        # ------------------- Q/K projections (permuted layout) -------------
        qfp, kfp = [], []
        for wname, bname, dest in (("wq_t", "bq", qfp), ("wk_t", "bk", kfp)):
            for it in range(NT):
                dt_ = big.tile([128, 8, 512], BF16, tag="big")
                dest.append(dt_)
                for g in range(2):
                    pss = [pmm.tile([128, 512], F32, tag="mm", name=f"mmps{g}_{i_}") for i_ in range(4)]
                    for kt in range(NT):
                        for jj in range(4):
                            w = 4 * g + jj
                            nc.tensor.matmul(
                                pss[jj],
                                lhsT=wsb[wname][:, kt, 128 * it : 128 * (it + 1)],
                                rhs=xnp[kt][:, w, :],
                                start=(kt == 0), stop=(kt == NT - 1),
                            )
                    for jj in range(4):
                        w = 4 * g + jj
                        nc.scalar.activation(
                            out=dt_[:, w, :], in_=pss[jj],
                            func=mybir.ActivationFunctionType.Identity,
                            bias=vec_sb[bname][:, it : it + 1], scale=1.0,
                        )

        # ------------------- scores (transposed) + exp ---------------------
        expst = []
        for dt in range(NT):
            ps = pmm.tile([128, 512], F32, tag="mm")
            idx = 0
            for j1 in range(8):
                for jt in range(NT):
                    nc.tensor.matmul(
                        ps,
                        lhsT=kfp[jt][:, j1, 128 * dt : 128 * (dt + 1)],
                        rhs=qfp[jt][:, j1, :],
                        start=(idx == 0), stop=(idx == 31),
                    )
                    idx += 1
            et_ = expool.tile([128, 512], BF16, tag="expst")
            nc.scalar.activation(
                out=et_, in_=ps, func=mybir.ActivationFunctionType.Exp, scale=SCALE
            )
            expst.append(et_)

        # ------------------- softmax denominators -> r ---------------------
        rsps = psm.tile([128, 512], F32, tag="small")
        for dt in range(NT):
            nc.tensor.matmul(
                rsps, lhsT=ones_sb, rhs=expst[dt], start=(dt == 0), stop=(dt == NT - 1)
            )
        rs_sb = rsp.tile([128, 512], F32, tag="rs")
        nc.scalar.copy(out=rs_sb, in_=rsps)
        rcol = rsp.tile([128, 4], F32, tag="rcol")
        for et in range(NT):
            nc.sync.dma_start(
                out=rcol[:, et : et + 1], in_=rs_sb[0:1, 128 * et : 128 * (et + 1)]
            )
        nc.vector.reciprocal(out=rcol, in_=rcol)

        # ------------------- V projection -> Vl ----------------------------
        vl = []
        for tt in range(NT):
            vt = big.tile([128, 8, 512], BF16, tag="big")
            vl.append(vt)
            for j1 in range(8):
                ps = pmm.tile([128, 512], F32, tag="mm")
                for kt in range(NT):
                    nc.tensor.matmul(
                        ps,
                        lhsT=xnp[kt][:, j1, 128 * tt : 128 * (tt + 1)],
                        rhs=wsb["wv_t"][:, kt, :],
                        start=(kt == 0), stop=False,
                    )
                nc.tensor.matmul(ps, lhsT=ones_sb, rhs=bvb_sb, start=False, stop=True)
                nc.scalar.copy(out=vt[:, j1, :], in_=ps)

        # ------------------- attention applied to V ------------------------
        attp = []
        for et in range(NT):
            at_ = big.tile([128, 8, 512], BF16, tag="big")
            attp.append(at_)
            for g in range(2):
                pss = [pmm.tile([128, 512], F32, tag="mm", name=f"mmps{g}_{i_}") for i_ in range(4)]
                for dt in range(NT):
                    for jj in range(4):
                        j1v = 4 * g + jj
                        nc.tensor.matmul(
                            pss[jj],
                            lhsT=expst[dt][:, 128 * et : 128 * (et + 1)],
                            rhs=vl[dt][:, j1v, :],
                            start=(dt == 0), stop=(dt == NT - 1),
                        )
                for jj in range(4):
                    j1v = 4 * g + jj
                    nc.scalar.activation(
                        out=at_[:, :, 64 * j1v : 64 * j1v + 64],
                        in_=pss[jj].rearrange("p (a r) -> p r a", r=8),
                        func=mybir.ActivationFunctionType.Copy,
                        scale=rcol[:, et : et + 1],
                    )

        # ------------------- out projection + residual ---------------------
        for ct in range(NT):
            for g in range(2):
                pss = [pmm.tile([128, 512], F32, tag="mm", name=f"mmps{g}_{i_}") for i_ in range(4)]
                for jj in range(4):
                    j1 = 4 * g + jj
                    for et in range(NT):
                        nc.tensor.matmul(
                            pss[jj],
                            lhsT=attp[et][:, j1, 128 * ct : 128 * (ct + 1)],
                            rhs=wsb["wo_t"][:, et, :],
                            start=(et == 0), stop=False,
                        )
                    nc.tensor.matmul(
                        pss[jj], lhsT=ones_sb, rhs=bob_sb, start=False, stop=True
                    )
                for jj in range(4):
                    j1 = 4 * g + jj
                    xr = xrp.tile([128, 512], F32, tag="xr")
                    nc.sync.dma_start(
                        out=xr,
                        in_=x_ap[s, 128 * ct : 128 * (ct + 1), 512 * j1 : 512 * (j1 + 1)],
                    )
                    yst = yp.tile([128, 512], F32, tag="y")
                    nc.vector.tensor_add(out=yst, in0=pss[jj], in1=xr)
                    nc.sync.dma_start(
                        out=y_ap[s, 128 * ct : 128 * (ct + 1), 512 * j1 : 512 * (j1 + 1)],
                        in_=yst,
                    )


def build_nc(repeat=1):
    key = ("nc", repeat)
    if key in _NC_CACHE:
        return _NC_CACHE[key]
    nc = bacc.Bacc("TRN2", target_bir_lowering=False, debug=False, num_devices=N_CORES)
    aps = {}
    aps["x"] = nc.dram_tensor("x", [SB, C, N], F32, kind="ExternalInput").ap()
    aps["y"] = nc.dram_tensor("y", [SB, C, N], F32, kind="ExternalOutput").ap()
    for wname in ("wq_t", "wk_t", "wv_t", "wo_t"):
        aps[wname] = nc.dram_tensor(wname, [C, C], BF16, kind="ExternalInput").ap()
    for vname in ("bq", "bk", "gn_w", "gn_b"):
        aps[vname] = nc.dram_tensor(vname, [C], F32, kind="ExternalInput").ap()
    aps["ones"] = nc.dram_tensor("ones", [128, 128], BF16, kind="ExternalInput").ap()
    aps["blkdiag"] = nc.dram_tensor("blkdiag", [128, 128], F32, kind="ExternalInput").ap()
    aps["bvb"] = nc.dram_tensor("bvb", [128, C], BF16, kind="ExternalInput").ap()
    aps["bob"] = nc.dram_tensor("bob", [128, C], BF16, kind="ExternalInput").ap()

    with tile.TileContext(nc) as tc:
        with (
            tc.tile_pool(name="const", bufs=1) as const,
            tc.tile_pool(name="xpool", bufs=2) as xpool,
            tc.tile_pool(name="big", bufs=16) as big,
            tc.tile_pool(name="expool", bufs=6) as expool,
            tc.tile_pool(name="spool", bufs=4) as spool,
            tc.tile_pool(name="rsp", bufs=2) as rsp,
            tc.tile_pool(name="xrp", bufs=3) as xrp,
            tc.tile_pool(name="yp", bufs=3) as yp,
            tc.tile_pool(name="pmm", bufs=6, space="PSUM") as pmm,
            tc.tile_pool(name="psm", bufs=2, space="PSUM") as psm,
        ):
            for rep in range(repeat):
                if rep:
                    # bench mode: keep repeats honest (no cross-repeat overlap)
                    tc.strict_bb_all_engine_barrier()
                _emit(nc, tc, aps, (const, xpool, big, expool, spool, rsp, xrp, yp, pmm, psm))
    nc.compile()
    _NC_CACHE[key] = nc
    return nc


# ---------------------------------------------------------------------------
# host side
# ---------------------------------------------------------------------------


def make_const_inputs(inputs):
    f32 = np.float32
    out = {
        "wq_t": np.ascontiguousarray(np.asarray(inputs["Wq"], f32).T).astype(BF_NP),
        "wk_t": np.ascontiguousarray(np.asarray(inputs["Wk"], f32).T).astype(BF_NP),
        "wv_t": np.ascontiguousarray(np.asarray(inputs["Wv"], f32).T).astype(BF_NP),
        "wo_t": np.ascontiguousarray(np.asarray(inputs["Wo"], f32).T).astype(BF_NP),
        "bq": np.asarray(inputs["bq"], f32),
        "bk": np.asarray(inputs["bk"], f32),
        "gn_w": np.asarray(inputs["gn_w"], f32),
        "gn_b": np.asarray(inputs["gn_b"], f32),
        "ones": np.ones((128, 128), BF_NP),
        "blkdiag": (np.kron(np.eye(8, dtype=f32), np.ones((16, 16), f32)) / 16.0).astype(f32),
        "bvb": np.ascontiguousarray(
            np.broadcast_to((np.asarray(inputs["bv"], f32) / 128.0).astype(BF_NP), (128, C))
        ),
        "bob": np.ascontiguousarray(
            np.broadcast_to((np.asarray(inputs["bo"], f32) / 128.0).astype(BF_NP), (128, C))
        ),
    }
    return out


def make_in_maps(inputs):
    x = np.asarray(inputs["x"], np.float32).reshape(B, C, N)
    const = make_const_inputs(inputs)
    in_maps = []
    for i in range(N_CORES):
        m = dict(const)
        m["x"] = np.ascontiguousarray(x[SB * i : SB * (i + 1)])
        in_maps.append(m)
    return in_maps


def kernel(**inputs) -> np.ndarray:
    nc = build_nc()
    in_maps = make_in_maps(inputs)
    res = run_bass_kernel_spmd(nc, in_maps, list(range(N_CORES)))
    out = np.empty((B, C, N), np.float32)
    for i in range(N_CORES):
        out[SB * i : SB * (i + 1)] = res.results[i]["y"]
    return out.reshape(B, C, H, W)


# ---------------------------------------------------------------------------
# cached PJRT runner (for repeat calls / on-device timing)
# ---------------------------------------------------------------------------


def pjrt_runner(repeat=1):
    """Build (once) a jitted 8-core runner over the compiled Bass module.

    Returns (fn, in_names, out_names). fn takes concat-along-axis-0 arrays
    (one per in_name, then one zero buffer per out_name) and returns concat
    outputs. Mirrors bass2jax.run_bass_via_pjrt but caches the jit so repeat
    calls skip re-tracing. With repeat=k the underlying Bass program runs the
    whole kernel k times back-to-back (barrier-separated) — used for timing.
    """
    rkey = ("runner", repeat)
    if rkey in _NC_CACHE:
        return _NC_CACHE[rkey]
    import jax
    from jax.sharding import Mesh, PartitionSpec
    from jax.experimental.shard_map import shard_map
    from concourse import bass2jax

    bass2jax.install_neuronx_cc_hook()
    nc = build_nc(repeat)
    assert nc.dbg_addr is None
    partition_name = nc.partition_id_tensor.name if nc.partition_id_tensor else None
    in_names, out_names, out_avals = [], [], []
    for alloc in nc.m.functions[0].allocations:
        if not isinstance(alloc, mybir.MemoryLocationSet):
            continue
        name = alloc.memorylocations[0].name
        if alloc.kind == "ExternalInput":
            if name != partition_name:
                in_names.append(name)
        elif alloc.kind == "ExternalOutput":
            out_names.append(name)
            out_avals.append(
                jax.core.ShapedArray(tuple(alloc.tensor_shape), mybir.dt.np(alloc.dtype))
            )

    all_names = tuple(in_names) + tuple(out_names)
    if partition_name is not None:
        all_names = all_names + (partition_name,)

    def _body(*args):
        operands = list(args)
        if partition_name is not None:
            operands.append(bass2jax.partition_id_tensor())
        outs = bass2jax._bass_exec_p.bind(
            *operands,
            out_avals=tuple(out_avals),
            in_names=all_names,
            out_names=tuple(out_names),
            lowering_input_output_aliases=(),
            sim_require_finite=True,
            sim_require_nnan=True,
            nc=nc,
        )
        return tuple(outs)

    devices = jax.devices()[:N_CORES]
    mesh = Mesh(np.asarray(devices), ("core",))
    nargs = len(in_names) + len(out_names)
    fn = jax.jit(
        shard_map(
            _body,
            mesh=mesh,
            in_specs=(PartitionSpec("core"),) * nargs,
            out_specs=(PartitionSpec("core"),) * len(out_names),
            check_rep=False,
        ),
        keep_unused=True,
    )
    _NC_CACHE[rkey] = (fn, in_names, out_names, out_avals, mesh)
    return _NC_CACHE[rkey]


if __name__ == "__main__":
    rng = np.random.default_rng(0)
    demo = {
        "x": rng.standard_normal((B, C, H, W), dtype=np.float32),
        "gn_w": np.ones((C,), np.float32),
        "gn_b": np.zeros((C,), np.float32),
    }
    for nm in ["Wq", "Wk", "Wv", "Wo"]:
        demo[nm] = (rng.standard_normal((C, C)) * 0.02).astype(np.float32)
    for nm in ["bq", "bk", "bv", "bo"]:
        demo[nm] = (rng.standard_normal((C,)) * 0.02).astype(np.float32)
    y = kernel(**demo)
    print("ok", y.shape, y.dtype)
